# revision 9
# baseline (speedup 1.0000x reference)
"""ArchSampler (ENAS-style controller) Trainium2 kernel, 8-way tensor parallel.

Self-contained: hardcodes all shapes. kernel(**inputs) takes the full
(unsharded) numpy inputs and returns (arch_seq int32 [64], sample_log_prob,
sample_entropy) matching reference.py.

Design:
- 2-layer LSTM (H=2048, no bias), 64 sequential steps, batch 1.
- Weights sharded 8 ways along the gate dimension (each core owns 256
  h-positions per layer => 1024 gate rows), converted to bf16 and
  pre-permuted on the host into the exact SBUF "moving operand" layout.
- Matvecs run with the WEIGHT as the moving operand (4x column-tiled
  TensorE => 512 weight elems/cycle/core); the input vector v is the
  stationary operand, stored as [128, 16] bf16 column tiles.
- Per-gate PSUM accumulation (col-group g computes gate g), so no partial
  reduction is needed; gates are DMA-reshaped [1,256]x4 -> [128, 8] for a
  fully partition-parallel LSTM cell.
- sigmoid(z) = (1 + tanh(z/2))/2 with the 0.5 pre-scales folded into the
  weights host-side (gates i,f,o) so only Tanh is used in the hot loop.
- h is carried as h2 = 2h (the 0.5 folded into consumer weight columns).
- Cross-core exchange of the tiny per-step vectors (h0, h1, x chunks,
  [128,2] bf16 each) uses remote_dma_broadcast point-to-point XOR mesh:
  core c sends its chunk to core c^d landing at slot d. Waits on the
  remote semaphores are injected into consumer instructions' sync_info
  AFTER Tile scheduling (Tile's single-core scheduling sim cannot model
  cross-core increments).
- Sampling: logits land as [32,1] (partition-major); argmax via 5-round
  XOR-shuffle max butterfly + is_equal (gumbel pre-jittered by -6e-8*type
  on host for first-index tie-breaking); idx / logit[idx] / sum(exp logit)
  are extracted with tiny ones/onehot matmuls, deferred to a batched
  epilogue so only Tanh's ACT table set is used inside the loop.
"""

import numpy as np

H = 2048
L = 2
T = 64
TY = 32
NCORES = 8
CH = H // NCORES  # 256 canonical chunk per core
GROWS = 4 * CH    # 1024 gate rows per core per layer

_cache = {}


def _f32(x):
    return np.ascontiguousarray(x, dtype=np.float32)


def _host_inputs(g_emb, w_emb, w_soft, w_ih, w_hh, gumbel):
    """Build per-core input maps (bf16 pre-permuted weights etc.)."""
    import ml_dtypes

    bf16 = ml_dtypes.bfloat16
    g_emb, w_emb, w_soft, w_ih, w_hh, gumbel = map(
        _f32, (g_emb, w_emb, w_soft, w_ih, w_hh, gumbel)
    )

    p = np.arange(128)
    kk = np.arange(16)
    s_of_k = kk // 2
    # HW-probed routing: sender q's Delta=d packet lands at core
    # q ^ d ^ (2 if d & 4 else 0), slot d. So receiver r's slot s holds the
    # canonical chunk of core r ^ s ^ (2 if s & 4 else 0).
    s_of_k = s_of_k ^ ((s_of_k & 4) >> 1)
    j_of_k = kk % 2
    gate_scale = np.array([0.5, 0.5, 1.0, 0.5], np.float32)  # i,f,g,o
    g4 = np.arange(4)
    n256 = np.arange(CH)

    # gumbel with tie-break jitter, [32, 64]
    gum = (gumbel.T - 6e-8 * np.arange(TY, dtype=np.float32)[:, None]).astype(
        np.float32
    )
    iota = np.arange(TY, dtype=np.float32).reshape(TY, 1).astype(bf16)
    ones = np.ones((TY, 1), np.float32).astype(bf16)

    in_maps = []
    for c in range(NCORES):
        # v-global index for (k, p): chunk (k//2 ^ c), element 2p + k%2
        vglob = 256 * (s_of_k[:, None] ^ c) + 2 * p[None, :] + j_of_k[:, None]  # [16,128]
        rows = 2048 * g4[:, None] + CH * c + n256[None, :]  # [4,256]

        wmov = np.empty((128, 2, 4, 2, 16, CH), np.float32)
        for l in range(L):
            for half in range(2):
                src = (w_ih, w_hh)[half][l]  # [8192, 2048]
                iscale = 1.0 if (l == 0 and half == 0) else 0.5  # h2=2h fold
                A = src[rows[:, :, None, None], vglob[None, None, :, :]]  # [4,256,16,128]
                A = A * (iscale * gate_scale[:, None, None, None])
                wmov[:, l, :, half, :, :] = A.transpose(3, 0, 2, 1)
        wmov = np.ascontiguousarray(wmov.reshape(128, 65536)).astype(bf16)

        # w_emb slice: [32, 64*256]; wemb[ty, t*256+n] = w_emb[t, ty, 256c+n]
        wemb = np.ascontiguousarray(
            w_emb[:, :, CH * c : CH * (c + 1)].transpose(1, 0, 2).reshape(TY, T * CH)
        ).astype(bf16)

        # w_soft: [128, 16*32]; wsoft[p, k*32+ty] = 0.5*w_soft[vglob[k,p], ty]
        ws = 0.5 * w_soft[vglob, :]  # [16, 128, 32]
        wsoft = np.ascontiguousarray(ws.transpose(1, 0, 2).reshape(128, 16 * TY)).astype(
            bf16
        )

        # x0: [128, 16]; x0[p, k] = g_emb[0, vglob[k, p]]
        x0 = np.ascontiguousarray(g_emb[0][vglob].T).astype(bf16)

        in_maps.append(
            {
                "wmov": wmov,
                "wemb": wemb,
                "wsoft": wsoft,
                "x0": x0,
                "gum": gum,
                "iota": iota,
                "ones": ones,
            }
        )
    return in_maps


def _add_wait(mybir, ins, sem, value):
    w = mybir.SyncWait(
        sync_type="semaphore",
        id=sem.num,
        ant_name=sem.name,
        wait_mode="sem-ge-imm",
        wait_value=value,
    )
    si = ins.sync_info
    if si is None:
        ins.sync_info = mybir.SyncInfo(on_wait=[w], on_update=[])
    else:
        ins.sync_info = mybir.SyncInfo(
            on_wait=list(si.on_wait) + [w], on_update=list(si.on_update)
        )


def _build_bass(nsteps=T):
    import concourse.bass as bass
    import concourse.mybir as mybir
    from concourse import tile, library_config, bacc

    f32 = mybir.dt.float32
    bf = mybir.dt.bfloat16
    AF = mybir.ActivationFunctionType
    OP = mybir.AluOpType

    nc = bass.Bass(target_bir_lowering=False)

    wmov_d = nc.declare_dram_parameter("wmov", [128, 65536], bf, isOutput=False)
    wemb_d = nc.declare_dram_parameter("wemb", [TY, T * CH], bf, isOutput=False)
    wsoft_d = nc.declare_dram_parameter("wsoft", [128, 16 * TY], bf, isOutput=False)
    x0_d = nc.declare_dram_parameter("x0", [128, 16], bf, isOutput=False)
    gum_d = nc.declare_dram_parameter("gum", [TY, T], f32, isOutput=False)
    iota_d = nc.declare_dram_parameter("iota", [TY, 1], bf, isOutput=False)
    ones_d = nc.declare_dram_parameter("ones", [TY, 1], bf, isOutput=False)
    out_idx_d = nc.declare_dram_parameter("out_idx", [T, 1], f32, isOutput=True)
    out_sc_d = nc.declare_dram_parameter("out_sc", [1, 2], f32, isOutput=True)

    rsem_h0 = nc.alloc_semaphore("rsem_h0")
    rsem_h1 = nc.alloc_semaphore("rsem_h1")
    rsem_x = nc.alloc_semaphore("rsem_x")
    lsem = nc.alloc_semaphore("lsem")

    deferred_waits = []  # (ins, sem, val)

    def broadcast(tl, rsem):
        """Send tl[:,0:2] to peer c^d at slot d, d=1..7, then trigger."""
        for d in range(1, 8):
            rdests = [None] * 8
            rdests[d] = (0, d)
            nc.gpsimd.remote_dma_broadcast(
                tl[:, 2 * d : 2 * d + 2],
                tl[:, 0:2],
                remote_sem=rsem,
                local_sem=lsem,
                rdests=rdests,
            )
        nc.gpsimd.trigger_dma(count=None)

    with tile.TileContext(nc) as tc:
        with (
            tc.tile_pool(name="wpool", bufs=1) as wpool,
            tc.tile_pool(name="spool", bufs=1) as spool,
            tc.tile_pool(name="wk", bufs=2) as wk,
            tc.tile_pool(name="pp", bufs=2, space="PSUM") as pp,
        ):
            wsb = wpool.tile([128, 65536], bf, tag="wsb", name="wsb")
            wemb = wpool.tile([TY, T * CH], bf, tag="wemb", name="wemb")
            wsoft = wpool.tile([128, 16 * TY], bf, tag="wsoft", name="wsoft")
            gum = wpool.tile([TY, T], f32, tag="gum", name="gum")
            iota = wpool.tile([TY, 1], bf, tag="iota", name="iota")
            ones = wpool.tile([TY, 1], bf, tag="ones", name="ones")

            xt = [spool.tile([128, 16], bf, tag=f"xt{i}", name=f"xt{i}") for i in range(2)]
            h0t = [spool.tile([128, 16], bf, tag=f"h0t{i}", name=f"h0t{i}") for i in range(2)]
            h1t = [spool.tile([128, 16], bf, tag=f"h1t{i}", name=f"h1t{i}") for i in range(2)]
            c0 = spool.tile([128, 2], f32, tag="c0", name="c0")
            c1 = spool.tile([128, 2], f32, tag="c1", name="c1")
            OH = spool.tile([TY, T], bf, tag="OH", name="OH")
            LB = spool.tile([TY, T], bf, tag="LB", name="LB")

            # ---- loads & init ----
            nc.sync.dma_start(out=wsb[:], in_=wmov_d[:])
            nc.sync.dma_start(out=wemb[:], in_=wemb_d[:])
            nc.sync.dma_start(out=wsoft[:], in_=wsoft_d[:])
            nc.sync.dma_start(out=xt[0][:], in_=x0_d[:])
            nc.sync.dma_start(out=gum[:], in_=gum_d[:])
            nc.sync.dma_start(out=iota[:], in_=iota_d[:])
            nc.sync.dma_start(out=ones[:], in_=ones_d[:])
            nc.vector.memset(h0t[1][:], 0.0)
            nc.vector.memset(h1t[1][:], 0.0)
            nc.vector.memset(c0[:], 0.0)
            nc.vector.memset(c1[:], 0.0)

            nc.gpsimd.load_library(library_config.remote_dma)

            def matvec(l, t):
                """Emit the 256 (LDW+MM) for layer l of step t. Returns psum tile."""
                pg = pp.tile([128, 256], f32, tag="gates", name="gates")
                if l == 0:
                    first_half = h0t[(t - 1) % 2]  # kk 16..31 emitted first: h-part
                    second_half = xt[t % 2]        # kk 0..15 emitted last: x-part
                    first_sem, first_val = rsem_h0, 14 * t
                    second_sem, second_val = rsem_x, 14 * t
                else:
                    first_half = h1t[(t - 1) % 2]  # h1_{t-1}
                    second_half = h0t[t % 2]       # h0_t (exchanged mid-step)
                    first_sem, first_val = rsem_h1, 14 * t
                    second_sem, second_val = rsem_h0, 14 * (t + 1)
                first_mm = [None, None]
                for phase, (src, base_kk) in enumerate(
                    ((first_half, 16), (second_half, 0))
                ):
                    for kk in range(16):
                        wk_idx = base_kk + kk
                        for g in range(4):
                            col0 = ((l * 4 + g) * 32 + wk_idx) * 256
                            mm = nc.tensor.matmul(
                                pg[32 * g : 32 * g + 1, :],
                                src[:, kk : kk + 1],
                                wsb[:, col0 : col0 + 256],
                                start=(phase == 0 and kk == 0),
                                stop=(phase == 1 and kk == 15),
                                tile_position=(0, 32 * g),
                            )
                            if first_mm[phase] is None:
                                first_mm[phase] = mm
                val_pairs = ((first_mm[0], first_sem, first_val),
                             (first_mm[1], second_sem, second_val))
                for mm, sem, val in val_pairs:
                    if val > 0:
                        deferred_waits.append((mm.ins, sem, val))
                return pg

            def cell(l, t, pg, cst, hdest):
                """LSTM cell elementwise from gates psum tile pg."""
                sg = wk.tile([128, 256], bf, tag=f"sg{l}", name=f"sg{l}")
                nc.vector.tensor_copy(sg[:], pg[:])
                G = wk.tile([128, 8], bf, tag=f"G{l}", name=f"G{l}")
                for g in range(4):
                    nc.sync.dma_start(
                        out=G[:, 2 * g : 2 * g + 2], in_=sg[32 * g : 32 * g + 1, :]
                    )
                Tt = wk.tile([128, 8], bf, tag=f"T{l}", name=f"T{l}")
                nc.scalar.activation(Tt[:], G[:], AF.Tanh)
                Ti, Tf, Tg, To = (Tt[:, 2 * i : 2 * i + 2] for i in range(4))
                sf = wk.tile([128, 2], bf, tag=f"sf{l}", name=f"sf{l}")
                nc.vector.tensor_scalar(sf[:], Tf, 0.5, 0.5, OP.mult, OP.add)
                si = wk.tile([128, 2], bf, tag=f"si{l}", name=f"si{l}")
                nc.vector.tensor_scalar(si[:], Ti, 0.5, 0.5, OP.mult, OP.add)
                A = wk.tile([128, 2], f32, tag=f"A{l}", name=f"A{l}")
                nc.vector.tensor_tensor(A[:], sf[:], cst[:], op=OP.mult)
                B = wk.tile([128, 2], f32, tag=f"B{l}", name=f"B{l}")
                nc.vector.tensor_tensor(B[:], si[:], Tg, op=OP.mult)
                nc.vector.tensor_tensor(cst[:], A[:], B[:], op=OP.add)
                TC = wk.tile([128, 2], bf, tag=f"TC{l}", name=f"TC{l}")
                nc.scalar.activation(TC[:], cst[:], AF.Tanh)
                U = wk.tile([128, 2], bf, tag=f"U{l}", name=f"U{l}")
                nc.vector.tensor_tensor(U[:], To, TC[:], op=OP.mult)
                # h2 = (1+t_o)*tanh(c) = 2h -> write into self slot
                nc.vector.tensor_tensor(hdest[:, 0:2], U[:], TC[:], op=OP.add)

            # ================== the 64 steps ==================
            for t in range(nsteps):
                pg0 = matvec(0, t)
                cell(0, t, pg0, c0, h0t[t % 2])
                broadcast(h0t[t % 2], rsem_h0)

                pg1 = matvec(1, t)
                cell(1, t, pg1, c1, h1t[t % 2])
                broadcast(h1t[t % 2], rsem_h1)

                # ---- logits [32,1] psum, consumes full h1t[t%2] ----
                pl = pp.tile([TY, 1], f32, tag="logits", name="logits")
                first_lg = None
                for k in range(16):
                    mm = nc.tensor.matmul(
                        pl[:],
                        wsoft[:, TY * k : TY * (k + 1)],
                        h1t[t % 2][:, k : k + 1],
                        start=(k == 0),
                        stop=(k == 15),
                    )
                    if first_lg is None:
                        first_lg = mm
                deferred_waits.append((first_lg.ins, rsem_h1, 14 * (t + 1)))

                # ---- z = logits + gumbel_t; butterfly max; onehot ----
                z = wk.tile([TY, 1], f32, tag="z", name="z")
                nc.vector.tensor_tensor(z[:], pl[:], gum[:, t : t + 1], op=OP.add)
                m = z
                for sbit in (1, 2, 4, 8, 16):
                    msh = wk.tile([TY, 1], f32, tag=f"msh{sbit}", name=f"msh{sbit}")
                    mask = [i ^ sbit for i in range(32)]
                    nc.vector.stream_shuffle(msh[:], m[:], mask)
                    m2 = wk.tile([TY, 1], f32, tag=f"m2{sbit}", name=f"m2{sbit}")
                    nc.vector.tensor_tensor(m2[:], m[:], msh[:], op=OP.max)
                    m = m2
                nc.vector.tensor_tensor(OH[:, t : t + 1], z[:], m[:], op=OP.is_equal)
                nc.vector.tensor_copy(LB[:, t : t + 1], pl[:])

                # ---- x_{t+1} gather + exchange (skip on last step) ----
                if t < nsteps - 1:
                    px = pp.tile([128, 2], f32, tag="xg", name="xg")
                    for j in range(2):
                        nc.tensor.matmul(
                            px[:, j : j + 1],
                            wemb[:, CH * t + j : CH * (t + 1) : 2],
                            OH[:, t : t + 1],
                            start=True,
                            stop=True,
                        )
                    nc.vector.tensor_copy(xt[(t + 1) % 2][:, 0:2], px[:])
                    broadcast(xt[(t + 1) % 2], rsem_x)

            # ================== epilogue ==================
            EZ = wk.tile([TY, T], bf, tag="EZ", name="EZ")
            nc.scalar.activation(EZ[:], LB[:], AF.Exp)
            SEL = wk.tile([TY, T], bf, tag="SEL", name="SEL")
            nc.vector.tensor_tensor(SEL[:], OH[:], LB[:], op=OP.mult)

            pS = pp.tile([1, T], f32, tag="gates", name="pS")
            nc.tensor.matmul(pS[:], ones[:], EZ[:], start=True, stop=True)
            pSel = pp.tile([1, T], f32, tag="logits", name="pSel")
            nc.tensor.matmul(pSel[:], ones[:], SEL[:], start=True, stop=True)
            pIdx = pp.tile([T, 1], f32, tag="xg", name="pIdx")
            nc.tensor.matmul(pIdx[:], OH[:], iota[:], start=True, stop=True)

            Ssb = wk.tile([1, T], f32, tag="Ssb", name="Ssb")
            nc.vector.tensor_copy(Ssb[:], pS[:])
            SELsb = wk.tile([1, T], f32, tag="SELsb", name="SELsb")
            nc.vector.tensor_copy(SELsb[:], pSel[:])
            IDXsb = wk.tile([T, 1], f32, tag="IDXsb", name="IDXsb")
            nc.vector.tensor_copy(IDXsb[:], pIdx[:])

            lS = wk.tile([1, T], f32, tag="lS", name="lS")
            nc.scalar.activation(lS[:], Ssb[:], AF.Ln)
            lp = wk.tile([1, T], f32, tag="lp", name="lp")
            nc.vector.tensor_tensor(lp[:], lS[:], SELsb[:], op=OP.subtract)
            elp = wk.tile([1, T], f32, tag="elp", name="elp")
            nc.scalar.activation(elp[:], lp[:], AF.Exp, scale=-1.0)
            ent = wk.tile([1, T], f32, tag="ent", name="ent")
            nc.vector.tensor_tensor(ent[:], lp[:], elp[:], op=OP.mult)
            sc = wk.tile([1, 2], f32, tag="sc", name="sc")
            nc.vector.tensor_reduce(
                sc[:, 0:1], lp[:, 0:nsteps], axis=mybir.AxisListType.X, op=OP.add
            )
            nc.vector.tensor_reduce(
                sc[:, 1:2], ent[:, 0:nsteps], axis=mybir.AxisListType.X, op=OP.add
            )
            nc.sync.dma_start(out=out_sc_d[:], in_=sc[:])
            nc.sync.dma_start(out=out_idx_d[:], in_=IDXsb[:])

    for ins, sem, val in deferred_waits:
        _add_wait(mybir, ins, sem, val)
    _hoist_library_reloads(nc)
    nc.compile()
    return nc


def _hoist_library_reloads(nc):
    for fn in nc.m.functions:
        for blk in fn.blocks:
            insts = blk.instructions
            idxs = [
                i
                for i, ins in enumerate(insts)
                if ins.__class__.__name__ == "InstPseudoReloadLibraryIndex"
            ]
            for k, i in enumerate(idxs):
                r = insts[i]
                del insts[i]
                insts.insert(k, r)


def _get_built(nsteps=T):
    key = ("nc", nsteps)
    if key not in _cache:
        _cache[key] = _build_bass(nsteps)
    return _cache[key]


def kernel(g_emb, w_emb, w_soft, w_ih, w_hh, gumbel):
    from concourse.bass_utils import run_bass_kernel_spmd

    in_maps = _host_inputs(g_emb, w_emb, w_soft, w_ih, w_hh, gumbel)
    nc = _get_built()
    res = run_bass_kernel_spmd(nc, in_maps, core_ids=list(range(NCORES)))
    out = res.results[0]
    arch_seq = np.rint(out["out_idx"].reshape(T)).astype(np.int32)
    lp_sum = np.float32(out["out_sc"][0, 0])
    ent_sum = np.float32(out["out_sc"][0, 1])
    return arch_seq, lp_sum, ent_sum


if __name__ == "__main__":
    import reference

    inputs = {k: np.asarray(v) for k, v in reference.setup_inputs().items()}
    got = kernel(**inputs)
    print("arch_seq:", got[0])
    print("lp:", got[1], "ent:", got[2])
